# revision 1
# baseline (speedup 1.0000x reference)
"""Trainium2 Bass kernel for the GNN attention module
(scatter-mean -> dense+tanh -> attention coefs -> weighted scatter-add),
data-parallel over graphs on 8 NeuronCores.

Self-contained: hardcodes N=2000000, D=64, G=8192, 8 cores.

Fused single-sweep design (per core, contiguous node/graph shard):
  For each 4096-node mega-tile t (nodes as [128 part, 32 blocks]):
    S1(t):  one-hot(graph-in-window) matmul -> transposed seg-sum [64, 32]
            accumulated into SBUF acc1 [64, GCP] at dynamic column offset.
    MID(t): after S1(t+1) (a graph spans at most 2 mega-tiles), window
            mean = acc1_win * invexp_win (1/counts precomputed on host),
            tgwin = tanh(W2^T @ mean) with W2=[W|W] duplicating the result
            into both partition halves for paired-block dots.
    S2(t):  dots = xts^T @ tgwin on PE; pick own-graph dot via one-hot
            mult + reduce; sigmoid -> coefs; coefs folded into one-hot;
            weighted seg-sum matmul -> acc2 [64, GCP].
  End: PE-transpose acc2 -> [GCP, 64], DMA out.
x is read once per mega-tile in row layout (xs) plus once in transposed
layout (xts) on a second DMA queue.
"""
import os
import numpy as np
from contextlib import ExitStack

import ml_dtypes

P = 128          # partitions / nodes per block
T = 32           # blocks per mega-tile
NT = P * T       # nodes per mega-tile (4096)
WIN = 32         # mega window width (graphs)
D = 64
DP2 = D + 2      # x columns + pad + packed per-block graph offset (b32)
N_FULL = 2_000_000
G_FULL = 8192
CORES = 8
GCP = 1152       # padded local graph count (9 * 128)
NCHUNK = GCP // P

LAST_EXEC_NS = None


# ----------------------------------------------------------------------------
# host-side preprocessing
# ----------------------------------------------------------------------------

def _shard_plan(batch, size, cores):
    counts = np.bincount(batch.astype(np.int64), minlength=size)
    cum = np.concatenate([[0], np.cumsum(counts)])
    n = batch.shape[0]
    gsplit = [0]
    for k in range(1, cores):
        g = int(np.searchsorted(cum, k * n / cores))
        g = max(gsplit[-1] + 1, min(g, size - (cores - k)))
        gsplit.append(g)
    gsplit.append(size)
    nsplit = [int(cum[g]) for g in gsplit]
    return gsplit, nsplit, counts


def _prep_core(x, batch, counts, g0, g1, n0, n1, n_meg):
    nn = n1 - n0
    npad = n_meg * NT
    lg = (batch[n0:n1] - g0).astype(np.int64)
    gc = g1 - g0
    ghost = gc                           # pad nodes get this local graph id
    lg_full = np.full(npad, ghost, dtype=np.int64)
    lg_full[:nn] = lg

    xs_pad = np.zeros((npad, D), dtype=np.float32)
    xs_pad[:nn] = x[n0:n1]

    lgt = lg_full.reshape(n_meg, T, P)            # [t, j, p]
    c0 = np.minimum(lgt[:, 0, 0], GCP - WIN)      # mega window base
    b32 = lgt - c0[:, None, None]
    assert b32.min() >= 0 and b32.max() < WIN, (b32.min(), b32.max())
    assert ghost + 1 <= GCP

    b32 = b32.transpose(0, 2, 1).astype(np.float32)   # [t, p, j]

    xs4 = np.zeros((n_meg, P, T, DP2), dtype=np.float32)
    xs4[:, :, :, :D] = xs_pad.reshape(n_meg, T, P, D).transpose(0, 2, 1, 3)
    xs4 = xs4.astype(ml_dtypes.bfloat16)
    xs4[:, :, :, D + 1] = b32.astype(ml_dtypes.bfloat16)
    xtb = xs_pad.reshape(n_meg, T, P, D).transpose(0, 1, 3, 2)  # [t, j, d, q]
    xts = np.ascontiguousarray(
        xtb.reshape(n_meg, T // 2, 2, D, P).transpose(0, 2, 3, 1, 4)
        .reshape(n_meg, P, (T // 2) * P)
    ).astype(ml_dtypes.bfloat16)

    c0s = np.zeros((1, n_meg), dtype=np.int32)
    c0s[0, :] = c0

    # 1 / max(counts, 1) for the local graphs, expanded to 64 partitions
    lc = np.ones(GCP, dtype=np.float64)
    lc[:gc] = np.maximum(counts[g0:g1], 1)
    inv = (1.0 / lc).astype(np.float32)
    invexp = np.ascontiguousarray(np.broadcast_to(inv, (D, GCP)))
    return {"xs": xs4, "xts": xts, "c0s": c0s, "invc": invexp}, gc


def _host_consts(W):
    iota = np.broadcast_to(
        np.arange(WIN, dtype=np.float32), (P, T, WIN)
    ).astype(ml_dtypes.bfloat16)
    ident = np.eye(P, dtype=np.float32)
    w2 = np.ascontiguousarray(
        np.tile(W.astype(np.float32), (1, 2))
    ).astype(ml_dtypes.bfloat16)          # [64, 128] = [W | W]
    return iota, ident, w2


# ----------------------------------------------------------------------------
# device kernel
# ----------------------------------------------------------------------------

def build_nc(n_meg):
    from concourse import mybir
    import concourse.tile as tile
    import concourse.bacc as bacc
    import concourse.bass as bass

    f32 = mybir.dt.float32
    bf16 = mybir.dt.bfloat16
    f16 = mybir.dt.float16
    i32 = mybir.dt.int32
    AF = mybir.ActivationFunctionType
    ALU = mybir.AluOpType
    ENG = mybir.EngineType

    nc = bacc.Bacc("TRN2", target_bir_lowering=False, debug=False,
                   num_devices=CORES)

    xs = nc.dram_tensor("xs", [n_meg, P, T, DP2], bf16, kind="ExternalInput").ap()
    xts = nc.dram_tensor("xts", [n_meg, P, (T // 2) * P], bf16, kind="ExternalInput").ap()
    c0s = nc.dram_tensor("c0s", [1, n_meg], i32, kind="ExternalInput").ap()
    w2c = nc.dram_tensor("w2c", [D, P], bf16, kind="ExternalInput").ap()
    invc = nc.dram_tensor("invc", [D, GCP], f32, kind="ExternalInput").ap()
    iotac = nc.dram_tensor("iotac", [P, T, WIN], bf16, kind="ExternalInput").ap()
    identc = nc.dram_tensor("identc", [P, P], f32, kind="ExternalInput").ap()
    out = nc.dram_tensor("out", [GCP, D], f32, kind="ExternalOutput").ap()

    with tile.TileContext(nc) as tc, ExitStack() as ctx:
        cpool = ctx.enter_context(tc.tile_pool(name="const", bufs=1))
        px = ctx.enter_context(tc.tile_pool(name="px", bufs=6))
        pxt = ctx.enter_context(tc.tile_pool(name="pxt", bufs=3))
        pbe = ctx.enter_context(tc.tile_pool(name="pbe", bufs=3))
        pmm = ctx.enter_context(tc.tile_pool(name="pmm", bufs=5))
        pm = ctx.enter_context(tc.tile_pool(name="pm", bufs=2))
        pk = ctx.enter_context(tc.tile_pool(name="pk", bufs=3))
        ptg = ctx.enter_context(tc.tile_pool(name="ptg", bufs=3))
        pmid = ctx.enter_context(tc.tile_pool(name="pmid", bufs=2))
        pp1 = ctx.enter_context(tc.tile_pool(name="pp1", bufs=3, space="PSUM"))
        ppd = ctx.enter_context(tc.tile_pool(name="ppd", bufs=2, space="PSUM"))
        ppw = ctx.enter_context(tc.tile_pool(name="ppw", bufs=1, space="PSUM"))

        iota_sb = cpool.tile([P, T, WIN], bf16)
        nc.sync.dma_start(iota_sb[:], iotac[:])
        ident_sb = cpool.tile([P, P], f32)
        nc.sync.dma_start(ident_sb[:], identc[:])
        w2_sb = cpool.tile([D, P], bf16)
        nc.sync.dma_start(w2_sb[:], w2c[:])
        inv_sb = cpool.tile([D, GCP], f32)
        nc.sync.dma_start(inv_sb[:], invc[:])
        c0_sb = cpool.tile([1, n_meg], i32)
        nc.sync.dma_start(c0_sb[:], c0s[:])

        acc1 = cpool.tile([D, GCP], f32)
        nc.vector.memset(acc1[:], 0.0)
        acc2 = cpool.tile([D, GCP], f32)
        nc.vector.memset(acc2[:], 0.0)
        nbias = cpool.tile([P, 1], f32)
        nc.vector.memset(nbias[:], -30.0)

        def c0_of(t, engines):
            return nc.values_load(
                c0_sb[0:1, t:t + 1], engines=engines,
                min_val=0, max_val=GCP - WIN, skip_runtime_bounds_check=True)

        xs_ts = [None] * n_meg
        m_ts = [None] * n_meg
        cv_ts = [None] * n_meg

        def S1(t):
            xs_t = px.tile([P, T, DP2], bf16, tag="xs")
            nc.sync.dma_start(xs_t[:], xs[t])
            b32e = pbe.tile([P, T, WIN], bf16, tag="b32e")
            nc.scalar.copy(b32e[:], xs_t[:, :, D + 1].to_broadcast([P, T, WIN]))
            m = pmm.tile([P, T, WIN], bf16, tag="M")
            nc.vector.tensor_tensor(out=m[:], in0=iota_sb[:], in1=b32e[:],
                                    op=ALU.is_equal)
            ps1 = pp1.tile([P, WIN], f32, tag="pacc")
            for jj in range(T // 2):
                nc.tensor.matmul(ps1[0:D, :], lhsT=xs_t[:, 2 * jj, 0:D],
                                 rhs=m[:, 2 * jj, :], tile_position=(0, 0),
                                 start=(jj == 0), stop=(jj == T // 2 - 1))
                nc.tensor.matmul(ps1[D:P, :], lhsT=xs_t[:, 2 * jj + 1, 0:D],
                                 rhs=m[:, 2 * jj + 1, :], tile_position=(0, 64),
                                 start=(jj == 0), stop=(jj == T // 2 - 1))
            cv = c0_of(t, engines=[ENG.DVE])
            a = acc1[:, bass.ds(cv, WIN)]
            nc.vector.tensor_tensor(out=a, in0=a, in1=ps1[0:D, :], op=ALU.add)
            nc.vector.tensor_tensor(out=a, in0=a, in1=ps1[D:P, :], op=ALU.add)
            xs_ts[t], m_ts[t], cv_ts[t] = xs_t, m, cv

        def MID(t):
            cv = cv_ts[t]
            meanT = pmid.tile([D, WIN], bf16, tag="meanT")
            nc.vector.tensor_tensor(out=meanT[:],
                                    in0=acc1[:, bass.ds(cv, WIN)],
                                    in1=inv_sb[:, bass.ds(cv, WIN)],
                                    op=ALU.mult)
            psW = ppw.tile([P, WIN], f32, tag="mid")
            nc.tensor.matmul(psW[:], lhsT=w2_sb[:], rhs=meanT[:],
                             start=True, stop=True)
            tgwin = ptg.tile([P, 2, WIN], bf16, tag="tgwin")
            nc.gpsimd.memset(tgwin[:], 0.0)
            nc.scalar.activation(tgwin[0:D, 0, :], psW[0:D, :], AF.Tanh)
            nc.scalar.activation(tgwin[D:P, 1, :], psW[D:P, :], AF.Tanh)
            return tgwin

        def S2(t, tgwin):
            xs_t, m, cv = xs_ts[t], m_ts[t], cv_ts[t]
            xts_t = pxt.tile([P, T // 2, P], bf16, tag="xts")
            nc.sync.dma_start(xts_t[:], xts[t])
            psD = ppd.tile([P, T, WIN], f32, tag="psD")
            for jj in range(T // 2):
                nc.tensor.matmul(psD[:, 2 * jj:2 * jj + 2, :],
                                 lhsT=xts_t[:, jj, :],
                                 rhs=tgwin[:, :, :],
                                 start=True, stop=True)
            # mp = sigmoid(dots)*onehot via saturation: sigma((dots+30)*m - 30)
            pick = pm.tile([P, T, WIN], f16, tag="pick")
            nc.vector.scalar_tensor_tensor(
                out=pick[:], in0=psD[:], scalar=30.0, in1=m[:],
                op0=ALU.add, op1=ALU.mult)
            mp = pm.tile([P, T, WIN], bf16, tag="Mp")
            nc.scalar.activation(mp[:], pick[:], AF.Sigmoid, bias=nbias[:])
            ps3 = pp1.tile([P, WIN], f32, tag="pacc")
            for jj in range(T // 2):
                nc.tensor.matmul(ps3[0:D, :], lhsT=xs_t[:, 2 * jj, 0:D],
                                 rhs=mp[:, 2 * jj, :], tile_position=(0, 0),
                                 start=(jj == 0), stop=(jj == T // 2 - 1))
                nc.tensor.matmul(ps3[D:P, :], lhsT=xs_t[:, 2 * jj + 1, 0:D],
                                 rhs=mp[:, 2 * jj + 1, :], tile_position=(0, 64),
                                 start=(jj == 0), stop=(jj == T // 2 - 1))
            a = acc2[:, bass.ds(cv, WIN)]
            nc.vector.tensor_tensor(out=a, in0=a, in1=ps3[0:D, :], op=ALU.add)
            nc.vector.tensor_tensor(out=a, in0=a, in1=ps3[D:P, :], op=ALU.add)
            xs_ts[t] = m_ts[t] = cv_ts[t] = None

        # ---------------- fused pipeline (lag-2: PE never waits) ------------
        S1(0)
        if n_meg > 1:
            S1(1)
        for t in range(n_meg):
            tgwin = MID(t)
            if t + 2 < n_meg:
                S1(t + 2)
            S2(t, tgwin)

        # ---------------- end: transpose acc2 -> out ------------------------
        for c in range(NCHUNK):
            pst = ppw.tile([P, D], f32, tag="mid")
            nc.tensor.transpose(pst[:], acc2[0:D, c * P:(c + 1) * P],
                                ident_sb[0:D, 0:D])
            oc = pk.tile([P, D], f32, tag="oc")
            nc.scalar.copy(oc[:], pst[:])
            nc.gpsimd.dma_start(out[c * P:(c + 1) * P, :], oc[:])

    nc.compile()
    return nc


# ----------------------------------------------------------------------------
# entry point
# ----------------------------------------------------------------------------

_CACHE = {}


def kernel(x, batch, size, W):
    global LAST_EXEC_NS
    from concourse import bass_utils

    x = np.asarray(x, dtype=np.float32)
    batch_np = np.asarray(batch).astype(np.int64)
    W = np.asarray(W, dtype=np.float32)
    size = int(size)
    cores = CORES

    gsplit, nsplit, counts = _shard_plan(batch_np, size, cores)
    max_nodes = max(nsplit[k + 1] - nsplit[k] for k in range(cores))
    n_meg = max(2, -(-max_nodes // NT))

    iota, ident, w2 = _host_consts(W)
    in_maps = []
    gcs = []
    for k in range(cores):
        m, gc = _prep_core(x, batch_np, counts, gsplit[k], gsplit[k + 1],
                           nsplit[k], nsplit[k + 1], n_meg)
        m["w2c"] = w2
        m["iotac"] = iota
        m["identc"] = ident
        in_maps.append(m)
        gcs.append(gc)

    if n_meg not in _CACHE:
        _CACHE[n_meg] = build_nc(n_meg)
    nc = _CACHE[n_meg]

    trace = os.environ.get("BASS_KERNEL_TRACE", "0") == "1"
    res = bass_utils.run_bass_kernel_spmd(nc, in_maps,
                                          core_ids=list(range(cores)),
                                          trace=trace)
    LAST_EXEC_NS = res.exec_time_ns
    outs = [res.results[k]["out"][:gcs[k]] for k in range(cores)]
    full = np.concatenate(outs, axis=0)
    if full.shape[0] < size:
        full = np.concatenate(
            [full, np.zeros((size - full.shape[0], D), np.float32)], axis=0)
    return np.ascontiguousarray(full[:size], dtype=np.float32)

